# revision 36
# baseline (speedup 1.0000x reference)
"""Trainium2 Bass kernel for nn_Autograd4bitQuantLinear (4-bit quant linear).

Computes out = x @ dequant4(qweight, scales, zeros) + bias where
  x:       (4, 2048, 4096) f32
  qweight: (512, 11008)    i32  (8 nibbles packed per int32 along rows)
  scales:  (11008, 1)      f32
  zeros:   (11008, 1)      f32
  bias:    (11008,)        f32
  out:     (4, 2048, 11008) f32

Strategy (tensor-parallel over 8 NeuronCores, column-sharded out_features):
  - Each core owns 1376 output columns; x is replicated.
  - Host-side layout prep: x is transposed to [k, m] and cast to bf16 once
    (shared across cores), with a k-permutation so that SBUF partition p of
    k-tile t = kg*8+j holds k = kg*1024 + 8p + j.  qweight is uploaded
    nibble-XORed with 0x88888888 so that a single sign-extending extract
    (qw2 << (28-4j)) >>arith 28 yields q-8 in [-8, 7] directly.
  - The matmul runs in fp8 (e4m3) DoubleRow mode at 2 k-tiles per MM:
    psum = sum_t x8 @ (q - 8).  q-8 is exact in e4m3; x is rounded
    bf16 -> e4m3 (ACT engine).  The affine dequant is recovered in the
    epilogue: out = psum*s + rowsum(x)*(8s - z) + bias, where rowsum(x)
    is computed at bf16 precision on the PE (ones-vector stationary
    matmuls over the bf16 x tiles) so the fp8 rounding error only enters
    through the centered weights (q-8).  Measured rel err ~1.8e-2.
  - Epilogue per (mt, n-chunk): t1 = coef*rs + b (STT), ob = ps*s (TT),
    ob += t1 (TT) on DVE; DMA out.
"""

import sys

sys.path.insert(0, "/opt/trn_rl_repo")

import numpy as np
import ml_dtypes

import concourse.bass as bass
import concourse.mybir as mybir
from concourse import bacc
from concourse.tile import TileContext


dt = mybir.dt
AL = mybir.AluOpType
DR = mybir.MatmulPerfMode.DoubleRow

P = 128
IN = 4096  # contraction dim (in_features)
OUT = 11008  # out_features
M_ROWS = 8192  # 4 * 2048
NCORES = 8
NSH = OUT // NCORES  # 1376 output columns per core
KT = IN // P  # 32 k-tiles
KT2 = KT // 2  # 16 fp8 pair-tiles (DoubleRow: 2 k-tiles per MM)
M_CHUNK = 512  # rows per x chunk
# n-chunks within the per-core shard; each must fit one PSUM bank (<=512 f32)
N_CHUNKS = ((0, 512), (512, 512), (1024, 352))
XT_BUFS = 44  # [128, M_CHUNK] bf16 x tiles resident (~1.4 chunks)
X8_BUFS = 24  # [128, 2, M_CHUNK] fp8 x tiles resident (~1.5 chunks)


def build(m_rows=M_ROWS, debug=False):
    """Build + compile the single-core Tile program (SPMD: same on all cores)."""
    assert m_rows % M_CHUNK == 0
    nc = bacc.Bacc(None, target_bir_lowering=False, debug=debug)

    xt_d = nc.dram_tensor("xt", [IN, m_rows], dt.bfloat16, kind="ExternalInput")
    qw_d = nc.dram_tensor("qw", [IN // 8, NSH], dt.int32, kind="ExternalInput")
    s_d = nc.dram_tensor("scales", [NSH], dt.float32, kind="ExternalInput")
    z_d = nc.dram_tensor("zeros", [NSH], dt.float32, kind="ExternalInput")
    b_d = nc.dram_tensor("bias", [NSH], dt.float32, kind="ExternalInput")
    out_d = nc.dram_tensor("out", [m_rows, NSH], dt.float32, kind="ExternalOutput")

    n_mchunks = m_rows // M_CHUNK
    mt_per_chunk = M_CHUNK // P

    with TileContext(nc) as tc:
        with (
            tc.tile_pool(name="singles", bufs=1) as singles,
            tc.tile_pool(name="w", bufs=KT2) as wpool,
            tc.tile_pool(name="qws", bufs=5) as qwpool,
            tc.tile_pool(name="nib", bufs=3) as nibpool,
            tc.tile_pool(name="xt", bufs=XT_BUFS) as xtpool,
            tc.tile_pool(name="x8", bufs=X8_BUFS) as x8pool,
            tc.tile_pool(name="ep", bufs=4) as eppool,
            tc.tile_pool(name="rs", bufs=2) as rspool,
            tc.tile_pool(name="rst", bufs=12) as rstpool,
            tc.tile_pool(name="rsd", bufs=2, space="DRAM") as rsdpool,
            tc.tile_pool(name="ps0", bufs=3, space="PSUM") as ps0pool,
            tc.tile_pool(name="ps", bufs=2, space="PSUM") as pspool,
            tc.tile_pool(name="rsp", bufs=1, space="PSUM") as rsppool,
        ):
            # ---- constants (broadcast over partitions) ----
            s_rep = singles.tile([P, NSH], dt.float32, tag="s_rep")
            nc.gpsimd.dma_start(out=s_rep[:], in_=s_d[None, :].to_broadcast([P, NSH]))
            z_rep = singles.tile([P, NSH], dt.float32, tag="z_rep")
            nc.gpsimd.dma_start(out=z_rep[:], in_=z_d[None, :].to_broadcast([P, NSH]))
            b_rep = singles.tile([P, NSH], dt.float32, tag="b_rep")
            nc.gpsimd.dma_start(out=b_rep[:], in_=b_d[None, :].to_broadcast([P, NSH]))
            # coef = 8*s - z
            coef_rep = singles.tile([P, NSH], dt.float32, tag="coef_rep")
            nc.vector.scalar_tensor_tensor(
                coef_rep[:], s_rep[:], 8.0, z_rep[:], AL.mult, AL.subtract
            )
            ones = singles.tile([P, 1], dt.bfloat16, tag="ones")
            nc.vector.memset(ones[:], 1.0)

            # ---- W dequant: w8(i, t2)[:, ko, :] = nibble(2*t2+ko) - 8, fp8 ----
            wtiles = {}  # (i, t2) -> [P, 2, w_i] fp8 tile

            qws_tiles = {}  # (i, kg) -> [P, w_i] int32 tile

            def load_qws():
                # all 12 slices up-front on the otherwise-idle gpsimd queue
                # (the scalar queue is busy with chunk-0 casts at startup)
                for i in range(len(N_CHUNKS)):
                    o, wd = N_CHUNKS[i]
                    for kg in range(KT // 8):
                        qws = qwpool.tile(
                            [P, wd], dt.int32, tag="qws", name=f"qws{i}_{kg}"
                        )
                        nc.gpsimd.dma_start(
                            out=qws[:], in_=qw_d[kg * P : (kg + 1) * P, o : o + wd]
                        )
                        qws_tiles[(i, kg)] = qws

            def unpack_group(i):
                o, wd = N_CHUNKS[i]
                for t2 in range(KT2):
                    wt = wpool.tile(
                        [P, 2, wd], dt.float8e4, tag=f"w{i}", name=f"w{i}_{t2}"
                    )
                    for ko in range(2):
                        t = 2 * t2 + ko
                        kg, j = t // 8, t % 8
                        qws = qws_tiles[(i, kg)]
                        # sign-extending nibble extract: (qw2<<(28-4j))>>a28 = q-8
                        # (bitvec TS can't cast, so extract i32 then cast-add)
                        nib = nibpool.tile([P, wd], dt.int32, tag="nib", name="nib")
                        nc.vector.tensor_scalar(
                            nib[:], qws[:], 28 - 4 * j, 28,
                            AL.logical_shift_left, AL.arith_shift_right,
                        )
                        nc.vector.tensor_scalar_add(wt[:, ko, :], nib[:], 0)
                    wtiles[(i, t2)] = wt

            def do_mm(ps, x8t, mt, t2, i):
                nc.tensor.matmul(
                    ps[:],
                    x8t[t2][:, :, mt * P : (mt + 1) * P],
                    wtiles[(i, t2)][:],
                    start=(t2 == 0),
                    stop=(t2 == KT2 - 1),
                    perf_mode=DR,
                )

            def epilogue_scale(ps, i):
                # psum-freeing op first so the next accumulation group's
                # start=True matmul isn't gated on the rest of the epilogue
                o, wd = N_CHUNKS[i]
                ob = eppool.tile([P, wd], dt.float32, tag=f"ob{i}", name=f"ob{i}")
                nc.vector.tensor_tensor(ob[:], ps[:], s_rep[:, o : o + wd], AL.mult)
                return ob

            def epilogue_finish(ob, row, i, rs_t):
                o, wd = N_CHUNKS[i]
                t1 = eppool.tile([P, 512], dt.float32, tag="t1", name="t1")
                nc.vector.scalar_tensor_tensor(
                    t1[:, :wd], coef_rep[:, o : o + wd], rs_t[:, 0:1],
                    b_rep[:, o : o + wd], AL.mult, AL.add,
                )
                nc.vector.tensor_tensor(ob[:], ob[:], t1[:, :wd], AL.add)
                nc.scalar.dma_start(out=out_d[row : row + P, o : o + wd], in_=ob[:])

            def load_chunk(mc):
                c0 = mc * M_CHUNK
                xts = []
                for t in range(KT):
                    xt = xtpool.tile([P, M_CHUNK], dt.bfloat16, tag="xt", name="xt")
                    nc.sync.dma_start(
                        out=xt[:], in_=xt_d[t * P : (t + 1) * P, c0 : c0 + M_CHUNK]
                    )
                    xts.append(xt)
                return xts

            def cast_chunk(xts):
                x8t = []
                for t2 in range(KT2):
                    x8 = x8pool.tile([P, 2, M_CHUNK], dt.float8e4, tag="x8", name="x8")
                    for ko in range(2):
                        nc.scalar.copy(x8[:, ko, :], xts[2 * t2 + ko][:])
                    x8t.append(x8)
                return x8t

            def rowsum_mm(psrs, xts, t):
                nc.tensor.matmul(
                    psrs[:], ones[:], xts[t][:],
                    start=(t == 0), stop=(t == KT - 1),
                )

            def rowsum_finish(psrs):
                rs_sb = rspool.tile([1, M_CHUNK], dt.float32, tag="rs_sb", name="rs_sb")
                nc.scalar.copy(rs_sb[:], psrs[:])  # noqa: rowsum tail
                rsd = rsdpool.tile([M_CHUNK, 1], dt.float32, tag="rsd", name="rsd")
                nc.sync.dma_start(out=rsd[:, 0:1], in_=rs_sb[0:1, :])
                rs_ts = []
                for mt in range(mt_per_chunk):
                    rs_t = rstpool.tile([P, 1], dt.float32, tag="rs_t", name="rs_t")
                    nc.sync.dma_start(
                        out=rs_t[:], in_=rsd[mt * P : (mt + 1) * P, 0:1]
                    )
                    rs_ts.append(rs_t)
                return rs_ts

            # ---- first m-chunk: n-chunk-major, interleaved with unpack ----
            xts0 = load_chunk(0)
            load_qws()
            x8t0 = cast_chunk(xts0)
            psrs0 = rsppool.tile([1, M_CHUNK], dt.float32, tag="rsp", name="rsp")
            for t in range(KT):
                rowsum_mm(psrs0, xts0, t)
            rs0 = rowsum_finish(psrs0)
            for i in range(len(N_CHUNKS)):
                unpack_group(i)
                for mt in range(mt_per_chunk):
                    g = i * mt_per_chunk + mt
                    pool_g = ps0pool if g % 3 == 0 else pspool
                    ps = pool_g.tile(
                        [P, N_CHUNKS[i][1]], dt.float32,
                        tag=f"ps{g % 3}", name=f"ps{g % 3}",
                    )
                    for t2 in range(KT2):
                        do_mm(ps, x8t0, mt, t2, i)
                    ob = epilogue_scale(ps, i)
                    epilogue_finish(ob, mt * P, i, rs0[mt])

            # ---- steady state ----
            for mc in range(1, n_mchunks):
                xts = load_chunk(mc)
                x8t = cast_chunk(xts)
                psrs = rsppool.tile([1, M_CHUNK], dt.float32, tag="rsp", name="rsp")
                rs = None
                for mt in range(mt_per_chunk):
                    pss = [
                        (ps0pool if i == 0 else pspool).tile(
                            [P, wd], dt.float32, tag=f"ps{i}", name=f"ps{i}"
                        )
                        for i, (o, wd) in enumerate(N_CHUNKS)
                    ]
                    for t2 in range(KT2):
                        for i in range(len(N_CHUNKS)):
                            do_mm(pss[i], x8t, mt, t2, i)
                        if mt == 0:
                            rowsum_mm(psrs, xts, 2 * t2)
                            rowsum_mm(psrs, xts, 2 * t2 + 1)
                    if mt == 0:
                        rs = rowsum_finish(psrs)
                    obs = [
                        epilogue_scale(pss[i], i) for i in range(len(N_CHUNKS))
                    ]
                    for i in range(len(N_CHUNKS)):
                        epilogue_finish(
                            obs[i], mc * M_CHUNK + mt * P, i, rs[mt]
                        )

    nc.compile()
    return nc


def _perm():
    """k-permutation: row t*P + p of xt holds x column kg*1024 + 8p + j."""
    perm = np.empty(IN, dtype=np.int64)
    ar = np.arange(P, dtype=np.int64)
    for t in range(KT):
        kg, j = t // 8, t % 8
        perm[t * P : (t + 1) * P] = kg * 8 * P + 8 * ar + j
    return perm


def make_in_maps(x2d, qweight, scales, zeros, bias):
    """Per-core input maps (host-side sharding / layout prep only)."""
    xt = np.ascontiguousarray(x2d.T[_perm()]).astype(ml_dtypes.bfloat16)
    # nibble-wise XOR with 8: packed nibbles become (q-8) mod 16 so the
    # device's sign-extending extract yields q-8 directly.
    qw2 = (qweight.view(np.uint32) ^ np.uint32(0x88888888)).view(np.int32)
    in_maps = []
    for c in range(NCORES):
        sl = slice(c * NSH, (c + 1) * NSH)
        in_maps.append(
            {
                "xt": xt,
                "qw": np.ascontiguousarray(qw2[:, sl]),
                "scales": np.ascontiguousarray(scales[sl, 0]),
                "zeros": np.ascontiguousarray(zeros[sl, 0]),
                "bias": np.ascontiguousarray(bias[sl]),
            }
        )
    return in_maps


_NC_CACHE = {}


def _get_nc(m_rows):
    if m_rows not in _NC_CACHE:
        _NC_CACHE[m_rows] = build(m_rows)
    return _NC_CACHE[m_rows]


def run_spmd(x2d, qweight, scales, zeros, bias, trace=False, **kwargs):
    """Run on the 8 NeuronCores; returns (out2d [8192, 11008] f32, results)."""
    from concourse.bass_utils import run_bass_kernel_spmd

    m_rows = x2d.shape[0]
    nc = _get_nc(m_rows)
    in_maps = make_in_maps(x2d, qweight, scales, zeros, bias)
    res = run_bass_kernel_spmd(
        nc, in_maps, list(range(NCORES)), trace=trace, **kwargs
    )
    outs = [res.results[c]["out"] for c in range(NCORES)]
    out2d = np.concatenate(outs, axis=1)
    return out2d, res


def kernel(x, qweight, scales, zeros, bias):
    x = np.asarray(x, dtype=np.float32)
    qweight = np.asarray(qweight, dtype=np.int32)
    scales = np.asarray(scales, dtype=np.float32)
    zeros = np.asarray(zeros, dtype=np.float32)
    bias = np.asarray(bias, dtype=np.float32)

    b, s, k_in = x.shape
    x2d = np.ascontiguousarray(x.reshape(b * s, k_in))
    out2d, _ = run_spmd(x2d, qweight, scales, zeros, bias)
    return out2d.reshape(b, s, OUT)


# revision 38
# speedup vs baseline: 1.0368x; 1.0368x over previous
"""Trainium2 Bass kernel for nn_Autograd4bitQuantLinear (4-bit quant linear).

Computes out = x @ dequant4(qweight, scales, zeros) + bias where
  x:       (4, 2048, 4096) f32
  qweight: (512, 11008)    i32  (8 nibbles packed per int32 along rows)
  scales:  (11008, 1)      f32
  zeros:   (11008, 1)      f32
  bias:    (11008,)        f32
  out:     (4, 2048, 11008) f32

Strategy (tensor-parallel over 8 NeuronCores, column-sharded out_features):
  - Each core owns 1376 output columns; x is replicated.
  - Host-side layout prep: x is transposed to [k, m] and cast to bf16 once
    (shared across cores), with a k-permutation so that SBUF partition p of
    k-tile t = kg*8+j holds k = kg*1024 + 8p + j.  qweight is uploaded
    nibble-XORed with 0x88888888 so that a single sign-extending extract
    (qw2 << (28-4j)) >>arith 28 yields q-8 in [-8, 7] directly.
  - The matmul runs in fp8 (e4m3) DoubleRow mode at 2 k-tiles per MM:
    psum = sum_t x8 @ (q - 8).  q-8 is exact in e4m3; x is rounded
    bf16 -> e4m3 (ACT engine).  The affine dequant is recovered in the
    epilogue: out = psum*s + rowsum(x)*(8s - z) + bias, where rowsum(x)
    is computed at bf16 precision on the PE (ones-vector stationary
    matmuls over the bf16 x tiles) so the fp8 rounding error only enters
    through the centered weights (q-8).  Measured rel err ~1.8e-2.
  - Epilogue per (mt, n-chunk): t1 = coef*rs + b (STT), ob = ps*s (TT),
    ob += t1 (TT) on DVE; DMA out.
"""

import sys

sys.path.insert(0, "/opt/trn_rl_repo")

import numpy as np
import ml_dtypes

import concourse.bass as bass
import concourse.mybir as mybir
from concourse import bacc
from concourse.tile import TileContext


dt = mybir.dt
AL = mybir.AluOpType
DR = mybir.MatmulPerfMode.DoubleRow

P = 128
IN = 4096  # contraction dim (in_features)
OUT = 11008  # out_features
M_ROWS = 8192  # 4 * 2048
NCORES = 8
NSH = OUT // NCORES  # 1376 output columns per core
KT = IN // P  # 32 k-tiles
KT2 = KT // 2  # 16 fp8 pair-tiles (DoubleRow: 2 k-tiles per MM)
M_CHUNK = 512  # rows per x chunk
# n-chunks within the per-core shard; each must fit one PSUM bank (<=512 f32)
N_CHUNKS = ((0, 512), (512, 512), (1024, 352))
XT_BUFS = 44  # [128, M_CHUNK] bf16 x tiles resident (~1.4 chunks)
X8_BUFS = 24  # [128, 2, M_CHUNK] fp8 x tiles resident (~1.5 chunks)


def build(m_rows=M_ROWS, debug=False):
    """Build + compile the single-core Tile program (SPMD: same on all cores)."""
    assert m_rows % M_CHUNK == 0
    nc = bacc.Bacc(None, target_bir_lowering=False, debug=debug)

    xt_d = nc.dram_tensor("xt", [IN, m_rows], dt.bfloat16, kind="ExternalInput")
    qw_d = nc.dram_tensor("qw", [IN // 8, NSH], dt.int32, kind="ExternalInput")
    s_d = nc.dram_tensor("scales", [NSH], dt.float32, kind="ExternalInput")
    z_d = nc.dram_tensor("zeros", [NSH], dt.float32, kind="ExternalInput")
    b_d = nc.dram_tensor("bias", [NSH], dt.float32, kind="ExternalInput")
    out_d = nc.dram_tensor("out", [m_rows, NSH], dt.float32, kind="ExternalOutput")

    n_mchunks = m_rows // M_CHUNK
    mt_per_chunk = M_CHUNK // P

    with TileContext(nc) as tc:
        with (
            tc.tile_pool(name="singles", bufs=1) as singles,
            tc.tile_pool(name="w", bufs=KT2) as wpool,
            tc.tile_pool(name="qws", bufs=6) as qwpool,
            tc.tile_pool(name="nib", bufs=4) as nibpool,
            tc.tile_pool(name="xt", bufs=XT_BUFS) as xtpool,
            tc.tile_pool(name="x8", bufs=X8_BUFS) as x8pool,
            tc.tile_pool(name="ep", bufs=4) as eppool,
            tc.tile_pool(name="rs", bufs=3) as rspool,
            tc.tile_pool(name="rst", bufs=12) as rstpool,
            tc.tile_pool(name="rsd", bufs=2, space="DRAM") as rsdpool,
            tc.tile_pool(name="ps0", bufs=3, space="PSUM") as ps0pool,
            tc.tile_pool(name="ps", bufs=2, space="PSUM") as pspool,
            tc.tile_pool(name="rsp", bufs=1, space="PSUM") as rsppool,
        ):
            # ---- constants (broadcast over partitions) ----
            s_rep = singles.tile([P, NSH], dt.float32, tag="s_rep")
            nc.gpsimd.dma_start(out=s_rep[:], in_=s_d[None, :].to_broadcast([P, NSH]))
            z_rep = singles.tile([P, NSH], dt.float32, tag="z_rep")
            nc.gpsimd.dma_start(out=z_rep[:], in_=z_d[None, :].to_broadcast([P, NSH]))
            b_rep = singles.tile([P, NSH], dt.float32, tag="b_rep")
            nc.gpsimd.dma_start(out=b_rep[:], in_=b_d[None, :].to_broadcast([P, NSH]))
            # coef = 8*s - z
            coef_rep = singles.tile([P, NSH], dt.float32, tag="coef_rep")
            nc.vector.scalar_tensor_tensor(
                coef_rep[:], s_rep[:], 8.0, z_rep[:], AL.mult, AL.subtract
            )
            ones = singles.tile([P, 1], dt.bfloat16, tag="ones")
            nc.vector.memset(ones[:], 1.0)

            # ---- W dequant: w8(i, t2)[:, ko, :] = nibble(2*t2+ko) - 8, fp8 ----
            wtiles = {}  # (i, t2) -> [P, 2, w_i] fp8 tile

            qws_tiles = {}  # (i, kg) -> [P, w_i] int32 tile

            def load_qws():
                # all 12 slices up-front on the otherwise-idle gpsimd queue
                # (the scalar queue is busy with chunk-0 casts at startup)
                for i in range(len(N_CHUNKS)):
                    o, wd = N_CHUNKS[i]
                    for kg in range(KT // 8):
                        qws = qwpool.tile(
                            [P, wd], dt.int32, tag="qws", name=f"qws{i}_{kg}"
                        )
                        nc.gpsimd.dma_start(
                            out=qws[:], in_=qw_d[kg * P : (kg + 1) * P, o : o + wd]
                        )
                        qws_tiles[(i, kg)] = qws

            def unpack_group(i):
                o, wd = N_CHUNKS[i]
                for t2 in range(KT2):
                    wt = wpool.tile(
                        [P, 2, wd], dt.float8e4, tag=f"w{i}", name=f"w{i}_{t2}"
                    )
                    for ko in range(2):
                        t = 2 * t2 + ko
                        kg, j = t // 8, t % 8
                        qws = qws_tiles[(i, kg)]
                        # sign-extending nibble extract: (qw2<<(28-4j))>>a28 = q-8
                        # (bitvec TS can't cast, so extract i32 then cast-add)
                        nib = nibpool.tile([P, wd], dt.int32, tag="nib", name="nib")
                        nc.vector.tensor_scalar(
                            nib[:], qws[:], 28 - 4 * j, 28,
                            AL.logical_shift_left, AL.arith_shift_right,
                        )
                        nc.vector.tensor_scalar_add(wt[:, ko, :], nib[:], 0)
                    wtiles[(i, t2)] = wt

            def do_mm(ps, x8t, mt, t2, i):
                nc.tensor.matmul(
                    ps[:],
                    x8t[t2][:, :, mt * P : (mt + 1) * P],
                    wtiles[(i, t2)][:],
                    start=(t2 == 0),
                    stop=(t2 == KT2 - 1),
                    perf_mode=DR,
                )

            def epilogue_scale(ps, i):
                # psum-freeing op first so the next accumulation group's
                # start=True matmul isn't gated on the rest of the epilogue
                o, wd = N_CHUNKS[i]
                ob = eppool.tile([P, wd], dt.float32, tag=f"ob{i}", name=f"ob{i}")
                nc.vector.tensor_tensor(ob[:], ps[:], s_rep[:, o : o + wd], AL.mult)
                return ob

            def epilogue_finish(ob, row, i, rs_t):
                o, wd = N_CHUNKS[i]
                t1 = eppool.tile([P, 512], dt.float32, tag="t1", name="t1")
                nc.vector.scalar_tensor_tensor(
                    t1[:, :wd], coef_rep[:, o : o + wd], rs_t[:, 0:1],
                    b_rep[:, o : o + wd], AL.mult, AL.add,
                )
                nc.vector.tensor_tensor(ob[:], ob[:], t1[:, :wd], AL.add)
                nc.scalar.dma_start(out=out_d[row : row + P, o : o + wd], in_=ob[:])

            def load_chunk(mc):
                c0 = mc * M_CHUNK
                xts = []
                for t in range(KT):
                    xt = xtpool.tile([P, M_CHUNK], dt.bfloat16, tag="xt", name="xt")
                    nc.sync.dma_start(
                        out=xt[:], in_=xt_d[t * P : (t + 1) * P, c0 : c0 + M_CHUNK]
                    )
                    xts.append(xt)
                return xts

            def cast_chunk(xts):
                x8t = []
                for t2 in range(KT2):
                    x8 = x8pool.tile([P, 2, M_CHUNK], dt.float8e4, tag="x8", name="x8")
                    for ko in range(2):
                        nc.scalar.copy(x8[:, ko, :], xts[2 * t2 + ko][:])
                    x8t.append(x8)
                return x8t

            def rowsum_chunk(xts):
                """rs[m] = sum_k x_bf16[k, m] via ones-stationary PE matmuls,
                round-tripped through DRAM to become per-partition scalars."""
                psrs = rsppool.tile([1, M_CHUNK], dt.float32, tag="rsp", name="rsp")
                for t in range(KT):
                    nc.tensor.matmul(
                        psrs[:], ones[:], xts[t][:],
                        start=(t == 0), stop=(t == KT - 1),
                    )
                rs_sb = rspool.tile([1, M_CHUNK], dt.float32, tag="rs_sb", name="rs_sb")
                nc.scalar.copy(rs_sb[:], psrs[:])
                rsd = rsdpool.tile([M_CHUNK, 1], dt.float32, tag="rsd", name="rsd")
                nc.sync.dma_start(out=rsd[:, 0:1], in_=rs_sb[0:1, :])
                rs_ts = []
                for mt in range(mt_per_chunk):
                    rs_t = rstpool.tile([P, 1], dt.float32, tag="rs_t", name="rs_t")
                    nc.sync.dma_start(
                        out=rs_t[:], in_=rsd[mt * P : (mt + 1) * P, 0:1]
                    )
                    rs_ts.append(rs_t)
                return rs_ts

            # ---- first m-chunk: n-chunk-major, interleaved with unpack ----
            xts0 = load_chunk(0)
            load_qws()
            x8t0 = cast_chunk(xts0)
            rs0 = rowsum_chunk(xts0)
            for i in range(len(N_CHUNKS)):
                unpack_group(i)
                for mt in range(mt_per_chunk):
                    g = i * mt_per_chunk + mt
                    pool_g = ps0pool if g % 3 == 0 else pspool
                    ps = pool_g.tile(
                        [P, N_CHUNKS[i][1]], dt.float32,
                        tag=f"ps{g % 3}", name=f"ps{g % 3}",
                    )
                    for t2 in range(KT2):
                        do_mm(ps, x8t0, mt, t2, i)
                    ob = epilogue_scale(ps, i)
                    epilogue_finish(ob, mt * P, i, rs0[mt])

            # ---- steady state ----
            for mc in range(1, n_mchunks):
                xts = load_chunk(mc)
                x8t = cast_chunk(xts)
                rs = rowsum_chunk(xts)
                for mt in range(mt_per_chunk):
                    pss = [
                        (ps0pool if i == 0 else pspool).tile(
                            [P, wd], dt.float32, tag=f"ps{i}", name=f"ps{i}"
                        )
                        for i, (o, wd) in enumerate(N_CHUNKS)
                    ]
                    for t2 in range(KT2):
                        for i in range(len(N_CHUNKS)):
                            do_mm(pss[i], x8t, mt, t2, i)
                    obs = [
                        epilogue_scale(pss[i], i) for i in range(len(N_CHUNKS))
                    ]
                    for i in range(len(N_CHUNKS)):
                        epilogue_finish(
                            obs[i], mc * M_CHUNK + mt * P, i, rs[mt]
                        )

    nc.compile()
    return nc


def _perm():
    """k-permutation: row t*P + p of xt holds x column kg*1024 + 8p + j."""
    perm = np.empty(IN, dtype=np.int64)
    ar = np.arange(P, dtype=np.int64)
    for t in range(KT):
        kg, j = t // 8, t % 8
        perm[t * P : (t + 1) * P] = kg * 8 * P + 8 * ar + j
    return perm


def make_in_maps(x2d, qweight, scales, zeros, bias):
    """Per-core input maps (host-side sharding / layout prep only)."""
    xt = np.ascontiguousarray(x2d.T[_perm()]).astype(ml_dtypes.bfloat16)
    # nibble-wise XOR with 8: packed nibbles become (q-8) mod 16 so the
    # device's sign-extending extract yields q-8 directly.
    qw2 = (qweight.view(np.uint32) ^ np.uint32(0x88888888)).view(np.int32)
    in_maps = []
    for c in range(NCORES):
        sl = slice(c * NSH, (c + 1) * NSH)
        in_maps.append(
            {
                "xt": xt,
                "qw": np.ascontiguousarray(qw2[:, sl]),
                "scales": np.ascontiguousarray(scales[sl, 0]),
                "zeros": np.ascontiguousarray(zeros[sl, 0]),
                "bias": np.ascontiguousarray(bias[sl]),
            }
        )
    return in_maps


_NC_CACHE = {}


def _get_nc(m_rows):
    if m_rows not in _NC_CACHE:
        _NC_CACHE[m_rows] = build(m_rows)
    return _NC_CACHE[m_rows]


def run_spmd(x2d, qweight, scales, zeros, bias, trace=False, **kwargs):
    """Run on the 8 NeuronCores; returns (out2d [8192, 11008] f32, results)."""
    from concourse.bass_utils import run_bass_kernel_spmd

    m_rows = x2d.shape[0]
    nc = _get_nc(m_rows)
    in_maps = make_in_maps(x2d, qweight, scales, zeros, bias)
    res = run_bass_kernel_spmd(
        nc, in_maps, list(range(NCORES)), trace=trace, **kwargs
    )
    outs = [res.results[c]["out"] for c in range(NCORES)]
    out2d = np.concatenate(outs, axis=1)
    return out2d, res


def kernel(x, qweight, scales, zeros, bias):
    x = np.asarray(x, dtype=np.float32)
    qweight = np.asarray(qweight, dtype=np.int32)
    scales = np.asarray(scales, dtype=np.float32)
    zeros = np.asarray(zeros, dtype=np.float32)
    bias = np.asarray(bias, dtype=np.float32)

    b, s, k_in = x.shape
    x2d = np.ascontiguousarray(x.reshape(b * s, k_in))
    out2d, _ = run_spmd(x2d, qweight, scales, zeros, bias)
    return out2d.reshape(b, s, OUT)
